# revision 8
# baseline (speedup 1.0000x reference)
"""DeltaNet forward on 8 TRN2 NeuronCores (Bass/Tile kernel).

Contract: kernel(**inputs) takes the FULL inputs of reference.setup_inputs()
and returns the FULL [B, L, D] output.

Sharding (hardcoded): 16 (batch, head) shards on 8 cores — core c handles
batch c//2 and heads {2*(c%2), 2*(c%2)+1}. Projection weights are sliced
per-core on the host; each core computes conv+silu for its batch, projects
q/k/v/beta/g for its two heads, runs the chunked delta rule (chunk=128,
(I+A)^-1 via Newton iteration — exact since A is strictly triangular and
nilpotent), applies the fused RMSNorm-swish gate and its slice of the output
projection. The host sums the two partial outputs per batch (tensor-parallel
output projection over heads).

Math notes:
 - q is NOT l2-normalized on device: each output row o[t] is linear in q[t],
   so the 1/||q[t]|| factor commutes through to the rms-norm, which is
   computed as o / sqrt(mean(o^2) + eps * ||q||^2)  — exact identity.
 - norm_w is folded into Wo on the host (diag scale on the contracted dim).
 - All matmuls use fp16 operands with fp32 PSUM accumulation.
"""

import os
import sys
import numpy as np

for _p in ("/opt/trn_rl_repo", "/root/.axon_site/_ro/trn_rl_repo"):
    if os.path.isdir(_p) and _p not in sys.path:
        sys.path.append(_p)

# ---- problem constants (hardcoded from the spec) ----
B, L, D = 4, 4096, 1024
H = 4
DK, DV = 512, 1024
DQH, DVH = DK // H, DV // H          # 128, 256
KC = 4                                # conv width
NORM_EPS = 1e-5
C = 128                               # delta-rule chunk (math-equiv to any)
SC = 512                              # superchunk (conv/dma granularity)
NEWTON_ITERS = 4                      # error = A^(2^5)=A^32; A^32 ~ 0 numerically

N_CORES = 8
TRACE = False                         # test.py flips this for profiling
TRACE_KW = {}
SIM_COMPAT = False                    # decompose Silu (CoreSim lacks it)

_BUILT = None


def _build(nc_L=L):
    import concourse.bass as bass
    import concourse.tile as tile
    import concourse.mybir as mybir
    from contextlib import ExitStack

    F32 = mybir.dt.float32
    F16 = mybir.dt.float16
    AF = mybir.ActivationFunctionType
    OP = mybir.AluOpType

    n_sc = nc_L // SC
    n_cc_per_sc = SC // C

    nc = bass.Bass("TRN2", target_bir_lowering=False, debug=False,
                   num_devices=N_CORES)

    xT = nc.dram_tensor("xT", [D, nc_L + KC - 1], F16, kind="ExternalInput").ap()
    wqk = nc.dram_tensor("wqk", [D, 512], F16, kind="ExternalInput").ap()
    wv = nc.dram_tensor("wv", [D, 512], F16, kind="ExternalInput").ap()
    wg = nc.dram_tensor("wg", [D, 512], F16, kind="ExternalInput").ap()
    wb = nc.dram_tensor("wb", [D, 2], F16, kind="ExternalInput").ap()
    wo = nc.dram_tensor("wo", [512, D], F16, kind="ExternalInput").ap()
    cw = nc.dram_tensor("cw", [D, KC], F32, kind="ExternalInput").ap()
    ident = nc.dram_tensor("ident", [128, 128], F16, kind="ExternalInput").ap()
    mtril = nc.dram_tensor("mtril", [128, 128], F32, kind="ExternalInput").ap()
    mtriu = nc.dram_tensor("mtriu", [128, 128], F32, kind="ExternalInput").ap()
    out_d = nc.dram_tensor("out", [nc_L, D], F32, kind="ExternalOutput").ap()

    KT = D // 128  # 8 K-tiles

    with tile.TileContext(nc) as tc, ExitStack() as ctx:
        wpool = ctx.enter_context(tc.tile_pool(name="w", bufs=1))
        xpool = ctx.enter_context(tc.tile_pool(name="x", bufs=2))
        hpool = ctx.enter_context(tc.tile_pool(name="h", bufs=2))
        cpool = ctx.enter_context(tc.tile_pool(name="cacc", bufs=4))
        spool = ctx.enter_context(tc.tile_pool(name="s", bufs=2))
        kpool = ctx.enter_context(tc.tile_pool(name="k", bufs=4))   # chunk tensors
        opool = ctx.enter_context(tc.tile_pool(name="o", bufs=2))   # out staging
        ps512 = ctx.enter_context(tc.tile_pool(name="ps512", bufs=3, space="PSUM"))
        pscc = ctx.enter_context(tc.tile_pool(name="pscc", bufs=3, space="PSUM"))
        psu = ctx.enter_context(tc.tile_pool(name="psu", bufs=2, space="PSUM"))

        # ---- constants / weights (resident) ----
        wqk_s = wpool.tile([128, KT, 512], F16, tag="wqk")
        nc.sync.dma_start(wqk_s[:], wqk.rearrange("(k p) c -> p k c", p=128))
        wv_s = wpool.tile([128, KT, 512], F16, tag="wv")
        nc.sync.dma_start(wv_s[:], wv.rearrange("(k p) c -> p k c", p=128))
        wg_s = wpool.tile([128, KT, 512], F16, tag="wg")
        nc.sync.dma_start(wg_s[:], wg.rearrange("(k p) c -> p k c", p=128))
        wb_s = wpool.tile([128, KT, 2], F16, tag="wb")
        nc.sync.dma_start(wb_s[:], wb.rearrange("(k p) c -> p k c", p=128))
        wo_s = wpool.tile([128, 4, D], F16, tag="wo")
        nc.sync.dma_start(wo_s[:], wo.rearrange("(j p) c -> p j c", p=128))
        cw_s = wpool.tile([128, KT, KC], F32, tag="cw")
        nc.sync.dma_start(cw_s[:], cw.rearrange("(k p) c -> p k c", p=128))
        id_s = wpool.tile([128, 128], F16, tag="id")
        nc.sync.dma_start(id_s[:], ident)
        mtril_s = wpool.tile([128, 128], F32, tag="mtril")
        nc.sync.dma_start(mtril_s[:], mtril)
        mtriu_s = wpool.tile([128, 128], F32, tag="mtriu")
        nc.sync.dma_start(mtriu_s[:], mtriu)

        # ---- per-head state ----
        S32 = []
        S16 = []
        for hh in range(2):
            s32 = spool.tile([128, DVH], F32, tag=f"S32_{hh}")
            s16 = spool.tile([128, DVH], F16, tag=f"S16_{hh}")
            nc.gpsimd.memset(s32[:], 0.0)
            nc.gpsimd.memset(s16[:], 0.0)
            S32.append(s32)
            S16.append(s16)

        for sc in range(n_sc):
            # ---- load x superchunk (pre-padded on host by KC-1 zeros) ----
            x_sb = xpool.tile([128, KT, SC + KC - 1], F16, tag="x")
            nc.sync.dma_start(
                x_sb[:],
                xT[:, sc * SC: sc * SC + SC + KC - 1].rearrange(
                    "(k p) l -> p k l", p=128),
            )
            # ---- causal depthwise conv + silu ----
            h_sb = hpool.tile([128, KT, SC], F16, tag="h")
            for k in range(KT):
                acc = cpool.tile([128, SC], F32, tag="cacc")
                nc.vector.tensor_scalar_mul(acc[:], x_sb[:, k, 0:SC],
                                            cw_s[:, k, 0:1])
                for i in (1, 2):
                    nc.vector.scalar_tensor_tensor(
                        acc[:], x_sb[:, k, i:i + SC], cw_s[:, k, i:i + 1],
                        acc[:], OP.mult, OP.add)
                acc2 = cpool.tile([128, SC], F32, tag="cacc2")
                nc.vector.scalar_tensor_tensor(
                    acc2[:], x_sb[:, k, 3:3 + SC], cw_s[:, k, 3:4],
                    acc[:], OP.mult, OP.add)
                if SIM_COMPAT:
                    sgm = cpool.tile([128, SC], F32, tag="csig")
                    nc.scalar.activation(sgm[:], acc2[:], AF.Sigmoid)
                    nc.vector.tensor_mul(h_sb[:, k, :], acc2[:], sgm[:])
                else:
                    nc.scalar.activation(h_sb[:, k, :], acc2[:], AF.Silu)

            for lc in range(n_cc_per_sc):
                cc = sc * n_cc_per_sc + lc
                tok = bass.ts(lc, C)

                def hT(k):
                    return h_sb[:, k, tok]

                # ---- projections (token-major) ----
                qk_ps = ps512.tile([128, 512], F32, tag="mm512")
                for k in range(KT):
                    nc.tensor.matmul(qk_ps[:], hT(k), wqk_s[:, k, :],
                                     start=(k == 0), stop=(k == KT - 1))
                v_ps = ps512.tile([128, 512], F32, tag="mm512")
                for k in range(KT):
                    nc.tensor.matmul(v_ps[:], hT(k), wv_s[:, k, :],
                                     start=(k == 0), stop=(k == KT - 1))
                g_ps = ps512.tile([128, 512], F32, tag="mm512")
                for k in range(KT):
                    nc.tensor.matmul(g_ps[:], hT(k), wg_s[:, k, :],
                                     start=(k == 0), stop=(k == KT - 1))
                b_ps = pscc.tile([128, 2], F32, tag="cc")
                for k in range(KT):
                    nc.tensor.matmul(b_ps[:], hT(k), wb_s[:, k, :],
                                     start=(k == 0), stop=(k == KT - 1))

                beta = kpool.tile([128, 2], F32, tag="beta")
                nc.scalar.activation(beta[:], b_ps[:], AF.Sigmoid)
                bneg = kpool.tile([128, 2], F32, tag="bneg")
                nc.vector.tensor_scalar_mul(bneg[:], beta[:], -1.0)

                sg16 = kpool.tile([128, 512], F16, tag="sg")
                if SIM_COMPAT:
                    gsg = kpool.tile([128, 512], F32, tag="gsig")
                    nc.scalar.activation(gsg[:], g_ps[:], AF.Sigmoid)
                    nc.vector.tensor_mul(sg16[:], g_ps[:], gsg[:])
                else:
                    nc.scalar.activation(sg16[:], g_ps[:], AF.Silu)

                q16 = kpool.tile([128, 256], F16, tag="q16")
                nc.scalar.copy(q16[:], qk_ps[:, 0:256])

                og16 = kpool.tile([128, 512], F16, tag="og")

                # ---- per-head precompute ----
                pre = []
                for hh in range(2):
                    qsl = slice(hh * 128, (hh + 1) * 128)
                    ksl = slice(256 + hh * 128, 256 + (hh + 1) * 128)
                    P = {}
                    # ||q||^2 (for the folded rms eps term)
                    scr16 = kpool.tile([128, 128], F16, tag="scr16")
                    sq_q = kpool.tile([128, 1], F32, tag="sq_q")
                    nc.scalar.activation(scr16[:], qk_ps[:, qsl], AF.Square,
                                         accum_out=sq_q[:])
                    P["sq_q"] = sq_q
                    # k l2 norm scale
                    scr32 = kpool.tile([128, 128], F16, tag="scr32")
                    sq_k = kpool.tile([128, 1], F32, tag="sq_k")
                    nc.scalar.activation(scr32[:], qk_ps[:, ksl], AF.Square,
                                         accum_out=sq_k[:])
                    nk = kpool.tile([128, 1], F32, tag="nk")
                    nc.scalar.activation(nk[:], sq_k[:], AF.Sqrt)
                    nk2 = kpool.tile([128, 1], F32, tag="nk2")
                    nc.vector.tensor_scalar_max(nk2[:], nk[:], 1e-12)
                    rs_k = kpool.tile([128, 1], F32, tag="rs_k")
                    nc.vector.reciprocal(rs_k[:], nk2[:])
                    # kn / kbn / bv
                    kn16 = kpool.tile([128, 128], F16, tag="kn")
                    nc.vector.tensor_scalar_mul(kn16[:], qk_ps[:, ksl], rs_k[:])
                    P["kn"] = kn16
                    kbn16 = kpool.tile([128, 128], F16, tag="kbn")
                    nc.vector.tensor_scalar_mul(kbn16[:], kn16[:],
                                                bneg[:, hh:hh + 1])
                    bv16 = kpool.tile([128, DVH], F16, tag="bv")
                    nc.vector.tensor_scalar_mul(
                        bv16[:], v_ps[:, hh * DVH:(hh + 1) * DVH],
                        beta[:, hh:hh + 1])
                    P["bv"] = bv16
                    # transposes
                    kT_ps = pscc.tile([128, 128], F16, tag="cc")
                    nc.tensor.transpose(kT_ps[:], kn16[:], id_s[:])
                    kT16 = kpool.tile([128, 128], F16, tag="kT")
                    nc.vector.tensor_copy(kT16[:], kT_ps[:])
                    qT_ps = pscc.tile([128, 128], F16, tag="cc")
                    nc.tensor.transpose(qT_ps[:], q16[:, qsl], id_s[:])
                    qT16 = kpool.tile([128, 128], F16, tag="qT")
                    nc.scalar.copy(qT16[:], qT_ps[:])
                    P["qT"] = qT16
                    # A = stril(beta_i * k_i.k_j) ; AT
                    araw_ps = pscc.tile([128, 128], F32, tag="cc")
                    nc.tensor.matmul(araw_ps[:], kT16[:], kT16[:],
                                     start=True, stop=True)
                    a16 = kpool.tile([128, 128], F16, tag="a16")
                    nc.vector.scalar_tensor_tensor(
                        a16[:], araw_ps[:], beta[:, hh:hh + 1], mtril_s[:],
                        OP.mult, OP.mult)
                    at_ps = pscc.tile([128, 128], F16, tag="cc")
                    nc.tensor.transpose(at_ps[:], a16[:], id_s[:])
                    at16 = kpool.tile([128, 128], F16, tag="at16")
                    nc.scalar.copy(at16[:], at_ps[:])
                    # Newton for X ~ (I+A)^-1 ; keep X and X^T
                    x16 = kpool.tile([128, 128], F16, tag="x16")
                    nc.gpsimd.tensor_sub(x16[:], id_s[:], a16[:])
                    xt16 = kpool.tile([128, 128], F16, tag="xt16")
                    nc.gpsimd.tensor_sub(xt16[:], id_s[:], at16[:])
                    mt16 = kpool.tile([128, 128], F16, tag="mt16")
                    nc.gpsimd.tensor_add(mt16[:], id_s[:], at16[:])
                    for it in range(NEWTON_ITERS):
                        t1_ps = pscc.tile([128, 128], F32, tag="cc")
                        nc.tensor.matmul(t1_ps[:], mt16[:], x16[:],
                                         start=True, stop=True)
                        t1n16 = kpool.tile([128, 128], F16, tag="t1n")
                        nc.scalar.mul(t1n16[:], t1_ps[:], -1.0)  # = -(M X)
                        t2t_ps = pscc.tile([128, 128], F32, tag="cc")
                        nc.tensor.matmul(t2t_ps[:], t1n16[:], xt16[:],
                                         start=True, stop=True)
                        xt_new = kpool.tile([128, 128], F16, tag="xt16")
                        nc.vector.scalar_tensor_tensor(
                            xt_new[:], xt16[:], 2.0, t2t_ps[:], OP.mult, OP.add)
                        if it < NEWTON_ITERS - 1:
                            t2_ps = pscc.tile([128, 128], F32, tag="cc")
                            nc.tensor.matmul(t2_ps[:], xt16[:], t1n16[:],
                                             start=True, stop=True)
                            x_new = kpool.tile([128, 128], F16, tag="x16")
                            nc.vector.scalar_tensor_tensor(
                                x_new[:], x16[:], 2.0, t2_ps[:], OP.mult, OP.add)
                            x16 = x_new
                        xt16 = xt_new
                    P["tinvT"] = xt16
                    # WcnT = -(Tinv kb)^T = kbn^T TinvT
                    wcn_ps = pscc.tile([128, 128], F32, tag="cc")
                    nc.tensor.matmul(wcn_ps[:], kbn16[:], xt16[:],
                                     start=True, stop=True)
                    wcn16 = kpool.tile([128, 128], F16, tag="wcn")
                    nc.vector.tensor_copy(wcn16[:], wcn_ps[:])
                    P["wcnT"] = wcn16
                    # GT masked = triu_incl_diag(kn q^T)
                    graw_ps = pscc.tile([128, 128], F32, tag="cc")
                    nc.tensor.matmul(graw_ps[:], kT16[:], qT16[:],
                                     start=True, stop=True)
                    gtm16 = kpool.tile([128, 128], F16, tag="gtm")
                    nc.vector.scalar_tensor_tensor(
                        gtm16[:], graw_ps[:], 1.0, mtriu_s[:], OP.mult, OP.mult)
                    P["gtm"] = gtm16
                    pre.append(P)

                # ---- sequential scan + gating, both heads ----
                for hh in range(2):
                    P = pre[hh]
                    u_ps = psu.tile([128, DVH], F32, tag="u256")
                    nc.tensor.matmul(u_ps[:], P["tinvT"][:], P["bv"][:],
                                     start=True, stop=False)
                    nc.tensor.matmul(u_ps[:], P["wcnT"][:], S16[hh][:],
                                     start=False, stop=True)
                    u16 = kpool.tile([128, DVH], F16, tag="u16")
                    nc.vector.tensor_copy(u16[:], u_ps[:])
                    o_ps = psu.tile([128, DVH], F32, tag="u256")
                    nc.tensor.matmul(o_ps[:], P["qT"][:], S16[hh][:],
                                     start=True, stop=False)
                    nc.tensor.matmul(o_ps[:], P["gtm"][:], u16[:],
                                     start=False, stop=True)
                    sd_ps = psu.tile([128, DVH], F32, tag="u256")
                    nc.tensor.matmul(sd_ps[:], P["kn"][:], u16[:],
                                     start=True, stop=True)
                    s32_new = spool.tile([128, DVH], F32, tag=f"S32_{hh}")
                    nc.vector.tensor_add(s32_new[:], S32[hh][:], sd_ps[:])
                    s16_new = spool.tile([128, DVH], F16, tag=f"S16_{hh}")
                    nc.scalar.copy(s16_new[:], s32_new[:])
                    S32[hh] = s32_new
                    S16[hh] = s16_new
                    # rms + gate:  og = o * rsc * silu(g)
                    scrv = kpool.tile([128, DVH], F16, tag="scrv")
                    ms = kpool.tile([128, 1], F32, tag="ms")
                    nc.scalar.activation(scrv[:], o_ps[:], AF.Square,
                                         accum_out=ms[:])
                    z = kpool.tile([128, 1], F32, tag="z")
                    nc.vector.scalar_tensor_tensor(
                        z[:], P["sq_q"][:], float(DVH * NORM_EPS), ms[:],
                        OP.mult, OP.add)
                    sqz = kpool.tile([128, 1], F32, tag="sqz")
                    nc.scalar.activation(sqz[:], z[:], AF.Sqrt,
                                         bias=0.0, scale=1.0 / DVH)
                    rsc = kpool.tile([128, 1], F32, tag="rsc")
                    nc.vector.reciprocal(rsc[:], sqz[:])
                    nc.vector.scalar_tensor_tensor(
                        og16[:, hh * DVH:(hh + 1) * DVH], o_ps[:], rsc[:],
                        sg16[:, hh * DVH:(hh + 1) * DVH], OP.mult, OP.mult)

                # ---- output projection (partial: this core's 2 heads) ----
                ogt = []
                for j in range(4):
                    ogt_ps = pscc.tile([128, 128], F16, tag="cc")
                    nc.tensor.transpose(ogt_ps[:], og16[:, bass.ts(j, 128)],
                                        id_s[:])
                    t = kpool.tile([128, 128], F16, tag=f"ogt{j}")
                    nc.scalar.copy(t[:], ogt_ps[:])
                    ogt.append(t)
                out32 = opool.tile([128, D], F32, tag="out32")
                for grp in range(2):
                    op_ps = ps512.tile([128, 512], F32, tag="mm512")
                    for j in range(4):
                        nc.tensor.matmul(op_ps[:], ogt[j][:],
                                         wo_s[:, j, bass.ts(grp, 512)],
                                         start=(j == 0), stop=(j == 3))
                    eng = nc.vector if grp == 0 else nc.scalar
                    if grp == 0:
                        nc.vector.tensor_copy(out32[:, bass.ts(grp, 512)],
                                              op_ps[:])
                    else:
                        nc.scalar.copy(out32[:, bass.ts(grp, 512)], op_ps[:])
                nc.sync.dma_start(out_d[cc * C:(cc + 1) * C, :], out32[:])

    return nc


def _split_drain_waits(nc):
    """Walrus enforces small per-instruction sync-wait capacities (1 for
    Drain, 2 observed-safe elsewhere). Hoist overflow waits onto preceding
    same-engine Drain instructions (1 wait each)."""
    import copy
    import concourse.mybir as mybir
    for f in nc.m.functions:
        for bb in f.blocks:
            new_insts = []
            for inst in bb.instructions:
                si = inst.sync_info
                limit = 1
                if si is not None and si.on_wait and len(si.on_wait) > limit:
                    waits = list(si.on_wait)
                    keep = waits[-limit:]
                    hoist = waits[:-limit]
                    for ci, w in enumerate(hoist):
                        d = mybir.InstDrain(
                            name=f"{inst.name}-ws{ci}",
                            ins=[], outs=[],
                            sync_info=mybir.SyncInfo(on_wait=[w], on_update=[]),
                        )
                        d.engine = inst.engine
                        new_insts.append(d)
                    inst.sync_info.on_wait = keep
                new_insts.append(inst)
            bb.instructions[:] = new_insts


def _get_built():
    global _BUILT
    if _BUILT is None:
        _BUILT = _build()
        _split_drain_waits(_BUILT)
    return _BUILT


def _prep_core_inputs(c, inputs):
    """Host-side sharding/layout prep for core c."""
    f16 = np.float16
    b = c // 2
    p = c % 2
    hs = [2 * p, 2 * p + 1]

    x = np.asarray(inputs["hidden_states"], np.float32)[b]        # [L, D]
    xT = np.zeros((D, L + KC - 1), np.float16)
    xT[:, KC - 1:] = x.T.astype(f16)

    Wq = np.asarray(inputs["Wq"], np.float32)
    Wk = np.asarray(inputs["Wk"], np.float32)
    Wv = np.asarray(inputs["Wv"], np.float32)
    Wb = np.asarray(inputs["Wb"], np.float32)
    Wg = np.asarray(inputs["Wg"], np.float32)
    Wo = np.asarray(inputs["Wo"], np.float32)
    norm_w = np.asarray(inputs["norm_w"], np.float32)
    conv_w = np.asarray(inputs["conv_w"], np.float32)

    qcols = np.concatenate([Wq[:, h * DQH:(h + 1) * DQH] for h in hs], axis=1)
    kcols = np.concatenate([Wk[:, h * DQH:(h + 1) * DQH] for h in hs], axis=1)
    wqk = np.concatenate([qcols, kcols], axis=1).astype(f16)       # [D, 512]
    wv = np.concatenate([Wv[:, h * DVH:(h + 1) * DVH] for h in hs],
                        axis=1).astype(f16)
    wg = np.concatenate([Wg[:, h * DVH:(h + 1) * DVH] for h in hs],
                        axis=1).astype(f16)
    wb = Wb[:, hs].astype(f16)
    Wo_eff = Wo * np.tile(norm_w, H)[:, None]
    wo = Wo_eff[p * 512:(p + 1) * 512, :].astype(f16)

    return {
        "xT": xT,
        "wqk": wqk, "wv": wv, "wg": wg, "wb": wb, "wo": wo,
        "cw": conv_w.astype(np.float32),
        "ident": np.eye(128, dtype=np.float16),
        "mtril": np.tril(np.ones((128, 128), np.float32), -1),
        "mtriu": np.triu(np.ones((128, 128), np.float32), 0),
    }


def kernel(**inputs):
    from concourse.bass_utils import run_bass_kernel_spmd

    nc = _get_built()
    in_maps = [_prep_core_inputs(c, inputs) for c in range(N_CORES)]
    res = run_bass_kernel_spmd(nc, in_maps, core_ids=list(range(N_CORES)),
                               trace=TRACE, **TRACE_KW)
    kernel.last_results = res
    out = np.zeros((B, L, D), np.float32)
    for b in range(B):
        out[b] = res.results[2 * b]["out"] + res.results[2 * b + 1]["out"]
    return out
